# revision 3
# baseline (speedup 1.0000x reference)
"""Trainium2 Bass kernel for nn_DiffusionPolicyHead (EDM/DDIM sampler head).

Strategy
--------
Pure data parallel over 8 NeuronCores (batch 32768 -> 4096/core).

Host-side algebra (per-step scalars fold into constants): with
    a_t = ratio + (1-ratio) c_skip,   b_t = (1-ratio) c_out,
the DDIM update is action' = a_t action + b_t (h3 @ Wout + bout).
Substituting action_t = g_t z_t + beta_t gives
    z_{t+1} = z_t + (b_t/g_{t+1}) * (h3 @ Wout),   z_0 = init_noise,
and layer 0 becomes  h0 = relu(z @ (c_in g W0a) + state @ W0s + e'_t)
with e'_t = emb_t @ W0e + b0 + c_in (beta_t @ W0a) a per-step bias vector.
Final output: action = g_50 z_50 + beta_50 (host).

Device layout (per core): feature-major [feat, batch]; batch 4096 = 2 halves
x 4 blocks of N=512 stacked in 32-partition strips of one [128, 512] z tile.
  - Layer 0: z matmuls are K=32 row-tiled (tile_position=(32*strip, 0)), two
    strips concurrent per PSUM pair-tile; state matmuls full K=128, f32r.
  - Hidden layers: fp8 e4m3 + DoubleRow (K=256 in one matmul, ~1.4x).
    Per-(step,layer) bias corrections calibrated on a 512-sample numpy run
    cancel the batch-coherent part of the fp8 quantization error.
  - Output layer: f32r, M=32 col-tiled (tile_position=(0, 32*strip)) so all
    4 blocks land in one PSUM bank -> single fused z-update DVE op.
Epilogues (relu+bias, fp32->fp8) merge 2 blocks per op ([128,1024] from two
adjacent PSUM banks) and alternate ACT/DVE to balance ~9 vs ~8 ops/half.
"""

import os
import sys

sys.path.insert(0, "/opt/trn_rl_repo")

import numpy as np
import ml_dtypes

BATCH, STATE_DIM, ACTION_DIM = 32768, 128, 32
HIDDEN, EMBED, N_STEPS = 256, 64, 50
SIGMA_MAX, SIGMA_MIN, RHO = 80.0, 0.001, 7.0
N_CORES = 8
B_CORE = BATCH // N_CORES  # 4096
NB = 512  # block columns (one PSUM bank of fp32)
N_HALF = 2  # halves per core; each half = 4 blocks in one [128, NB] z tile
CAL_SAMPLES = 512

_cached = {}


def _q8(x):
    return np.asarray(x, ml_dtypes.float8_e4m3).astype(np.float32)


def _r11(x):
    """float32r as seen by the PE: mantissa 23->11 bits, RNE."""
    u = np.ascontiguousarray(np.asarray(x, np.float32)).view(np.uint32)
    half = np.uint32(1 << 11)
    u = u + (half - 1 + ((u >> 12) & 1))
    u &= np.uint32(0xFFFFF000)
    return u.view(np.float32)


def _host_tables(W0, b0, bout):
    """Fold per-step diffusion constants into weight tables (float64)."""
    W0 = W0.astype(np.float64)
    b0 = b0.astype(np.float64)
    bout = bout.astype(np.float64)
    W0a = W0[:ACTION_DIM]
    W0e = W0[ACTION_DIM : ACTION_DIM + EMBED]
    W0s = W0[ACTION_DIM + EMBED :]

    ramp = np.linspace(0.0, 1.0, N_STEPS)
    min_r, max_r = SIGMA_MIN ** (1.0 / RHO), SIGMA_MAX ** (1.0 / RHO)
    sig = np.concatenate([(max_r + ramp * (min_r - max_r)) ** RHO, np.zeros(1)])

    half = EMBED // 2
    freqs = np.exp(-np.log(10000.0) * np.arange(half, dtype=np.float64) / half)

    sd = 1.0
    g = sig[0]
    beta = np.zeros(ACTION_DIM)
    W0A = np.empty((N_STEPS, ACTION_DIM, HIDDEN))
    eprime = np.empty((N_STEPS, HIDDEN))
    s_t = np.empty(N_STEPS)
    for t in range(N_STEPS):
        s, sn = sig[t], sig[t + 1]
        var = s * s + sd * sd
        c_in = 1.0 / np.sqrt(var)
        c_skip = sd * sd / var
        c_out = s * sd / np.sqrt(var)
        ratio = sn / s
        a_t = ratio + (1.0 - ratio) * c_skip
        b_t = (1.0 - ratio) * c_out
        ang = np.log(s) * freqs
        emb = np.concatenate([np.sin(ang), np.cos(ang)])
        W0A[t] = c_in * g * W0a
        eprime[t] = emb @ W0e + b0 + c_in * (beta @ W0a)
        g_next = a_t * g
        beta = a_t * beta + b_t * bout
        s_t[t] = b_t / g_next
        g = g_next
    return dict(
        W0A=W0A.astype(np.float32),
        eprime=eprime.astype(np.float32),
        s_t=s_t.astype(np.float32),
        W0s=W0s.astype(np.float32),
        g_final=g,
        beta_final=beta,
    )


def _calibrate_full(state, init_noise, Wh, bh, Wout, tb, n_steps):
    """Per-(step, layer) bias corrections for fp8 hidden layers: batch-mean
    of (fp8 matmul - exact matmul) on a sample, emulating device rounding."""
    rng = np.random.default_rng(12345)
    idx = rng.choice(BATCH, CAL_SAMPLES, replace=False)
    st = state[idx].astype(np.float32)
    z = init_noise[idx].astype(np.float32).copy()
    u = _r11(st) @ _r11(tb["W0s"])
    Wh8 = _q8(Wh)
    Wout_r = _r11(Wout)
    db = np.zeros((n_steps, 3, HIDDEN), np.float32)
    for t in range(n_steps):
        h = np.maximum(_r11(z) @ _r11(tb["W0A"][t]) + u + tb["eprime"][t], 0.0)
        for l in range(3):
            A = _q8(h) @ Wh8[l]
            db[t, l] = -(A - h @ Wh[l]).mean(axis=0)
            h = np.maximum(A + bh[l] + db[t, l], 0.0)
        z = z + tb["s_t"][t] * (_r11(h) @ Wout_r)
    return db


def _build_program(n_steps, s_t):
    import concourse.bacc as bacc
    import concourse.mybir as mybir
    from concourse import tile
    from contextlib import ExitStack

    F32 = mybir.dt.float32
    F32R = mybir.dt.float32r
    F8 = mybir.dt.float8e4
    AF = mybir.ActivationFunctionType
    ALU = mybir.AluOpType
    DR = mybir.MatmulPerfMode.DoubleRow

    nc = bacc.Bacc("TRN2", target_bir_lowering=False, debug=False, num_devices=N_CORES)

    state_in = nc.declare_dram_parameter("stateT", [STATE_DIM, B_CORE], F32R, isOutput=False)
    zinit_in = nc.declare_dram_parameter("zinit", [N_HALF, 128, NB], F32R, isOutput=False)
    wtab_in = nc.declare_dram_parameter("WTAB", [n_steps, 128, HIDDEN], F32R, isOutput=False)
    btab_in = nc.declare_dram_parameter("BTAB", [n_steps, 128, 8], F32, isOutput=False)
    w0s_in = nc.declare_dram_parameter("W0s", [STATE_DIM, HIDDEN], F32R, isOutput=False)
    wh8_in = nc.declare_dram_parameter("WH8", [128, 3, 2, 2, 128], F8, isOutput=False)
    wout_in = nc.declare_dram_parameter("WOUT", [128, 2, ACTION_DIM], F32R, isOutput=False)
    out_ext = nc.declare_dram_parameter("outZ", [N_HALF, 128, NB], F32R, isOutput=True)

    with tile.TileContext(nc) as tc:
        with ExitStack() as ctx:
            wpool = ctx.enter_context(tc.tile_pool(name="weights", bufs=1))
            zpool = ctx.enter_context(tc.tile_pool(name="zbufs", bufs=1))
            hpool = ctx.enter_context(tc.tile_pool(name="acts8", bufs=6))
            h3pool = ctx.enter_context(tc.tile_pool(name="acts3", bufs=4))
            wstream = ctx.enter_context(tc.tile_pool(name="wstream", bufs=4))
            bstream = ctx.enter_context(tc.tile_pool(name="bstream", bufs=4))
            ppool = ctx.enter_context(tc.tile_pool(name="psum", bufs=4, space="PSUM"))

            stateT = wpool.tile([STATE_DIM, B_CORE], F32R, tag="stateT")
            w0s = wpool.tile([STATE_DIM, HIDDEN], F32R, tag="w0s")
            wh8 = wpool.tile([128, 3, 2, 2, 128], F8, tag="wh8")
            wout = wpool.tile([128, 2, ACTION_DIM], F32R, tag="wout")
            for b in range(8):
                nc.sync.dma_start(
                    stateT[:, b * NB : (b + 1) * NB],
                    state_in[:, b * NB : (b + 1) * NB],
                )
            nc.sync.dma_start(w0s[:], w0s_in[:])
            nc.sync.dma_start(wh8[:], wh8_in[:])
            nc.sync.dma_start(wout[:], wout_in[:])

            # z ping-pong: [2 parity][2 halves] tiles [128, 512], 4 blocks
            # per tile in 32-row strips. Only parity 0 needs init (parity 1
            # is fully written by step 0's z-update).
            zt = [
                [
                    zpool.tile([128, NB], F32R, tag=f"z{p}_{h}", name=f"z{p}_{h}")
                    for h in range(N_HALF)
                ]
                for p in range(2)
            ]
            for h in range(N_HALF):
                nc.sync.dma_start(zt[0][h][:], zinit_in[h])

            for t in range(n_steps):
                wtab = wstream.tile([128, HIDDEN], F32R, tag="wtab", name="wtab")
                btab = bstream.tile([128, 8], F32, tag="btab", name="btab")
                nc.sync.dma_start(wtab[:], wtab_in[t])
                nc.sync.dma_start(btab[:], btab_in[t])

                for half in range(N_HALF):
                    zc, zn = zt[t % 2][half], zt[(t + 1) % 2][half]
                    h_cur = []  # per pair tiles of current layer
                    # ---- layer 0 ----
                    for pair in range(2):
                        h0 = hpool.tile(
                            [128, 2, 2, NB], F8, tag=f"h0_{pair}", name=f"h0_{pair}"
                        )
                        for j in range(2):
                            jsl = slice(j * 128, (j + 1) * 128)
                            pt = ppool.tile([128, 2 * NB], F32, tag="ps", name="p0")
                            for bp in range(2):
                                strip = pair * 2 + bp
                                gcol = (half * 4 + strip) * NB
                                nc.tensor.matmul(
                                    pt[:, bp * NB : (bp + 1) * NB],
                                    w0s[:, jsl],
                                    stateT[:, gcol : gcol + NB],
                                    start=True,
                                    stop=False,
                                )
                            for bp in range(2):
                                strip = pair * 2 + bp
                                ssl = slice(strip * 32, (strip + 1) * 32)
                                nc.tensor.matmul(
                                    pt[:, bp * NB : (bp + 1) * NB],
                                    wtab[ssl, jsl],
                                    zc[ssl, :],
                                    start=False,
                                    stop=True,
                                    tile_position=(strip * 32, 0),
                                )
                            bias_ap = btab[:, j : j + 1]
                            if j == 0:
                                nc.scalar.activation(
                                    h0[:, :, j, :], pt[:], AF.Relu, bias=bias_ap
                                )
                            else:
                                nc.vector.tensor_scalar(
                                    h0[:, :, j, :], pt[:], bias_ap, 0.0, ALU.add, ALU.max
                                )
                        h_cur.append(h0)
                    # ---- hidden layers (fp8 DoubleRow) ----
                    for l in range(3):
                        last = l == 2
                        h_next = []
                        for pair in range(2):
                            if last:
                                hn = h3pool.tile(
                                    [128, 2, 2, NB],
                                    F32R,
                                    tag=f"h3_{pair}",
                                    name=f"h3_{pair}",
                                )
                            else:
                                hn = hpool.tile(
                                    [128, 2, 2, NB],
                                    F8,
                                    tag=f"h{l + 1}_{pair}",
                                    name=f"h{l + 1}_{pair}",
                                )
                            for j in range(2):
                                pt = ppool.tile([128, 2 * NB], F32, tag="ps", name="pl")
                                for bp in range(2):
                                    nc.tensor.matmul(
                                        pt[:, bp * NB : (bp + 1) * NB],
                                        wh8[:, l, j, :, :],
                                        h_cur[pair][:, bp, :, :],
                                        start=True,
                                        stop=True,
                                        perf_mode=DR,
                                    )
                                bias_ap = btab[:, 2 + l * 2 + j : 3 + l * 2 + j]
                                # ACT 9 / DVE 7 ops per half: j=0 on ACT,
                                # j=1 on DVE, except L3 pair1 j=1 on ACT.
                                on_act = j == 0 or (last and pair == 1)
                                if on_act:
                                    nc.scalar.activation(
                                        hn[:, :, j, :], pt[:], AF.Relu, bias=bias_ap
                                    )
                                else:
                                    nc.vector.tensor_scalar(
                                        hn[:, :, j, :],
                                        pt[:],
                                        bias_ap,
                                        0.0,
                                        ALU.add,
                                        ALU.max,
                                    )
                            h_next.append(hn)
                        h_cur = h_next
                    # ---- output layer (f32r, col-tiled M=32) ----
                    po = ppool.tile([128, 2 * NB], F32, tag="ps", name="po")
                    for c in range(2):
                        for strip in range(4):
                            pair, bp = strip // 2, strip % 2
                            nc.tensor.matmul(
                                po[strip * 32 : (strip + 1) * 32, :NB],
                                wout[:, c, :],
                                h_cur[pair][:, bp, c, :],
                                start=(c == 0),
                                stop=(c == 1),
                                tile_position=(0, strip * 32),
                            )
                    nc.vector.scalar_tensor_tensor(
                        zn[:, :],
                        po[:, :NB],
                        float(s_t[t]),
                        zc[:, :],
                        ALU.mult,
                        ALU.add,
                    )

            zfin = zt[n_steps % 2]
            for h in range(N_HALF):
                nc.sync.dma_start(out_ext[h], zfin[h][:, :])

    nc.compile()
    return nc


def kernel(state, init_noise, W0, b0, Wh, bh, Wout, bout):
    from concourse.bass_utils import run_bass_kernel_spmd

    state = np.ascontiguousarray(np.asarray(state, np.float32))
    init_noise = np.ascontiguousarray(np.asarray(init_noise, np.float32))
    Wh_np = np.asarray(Wh, np.float32)
    bh_np = np.asarray(bh, np.float32)
    Wout_np = np.asarray(Wout, np.float32)

    tb = _host_tables(np.asarray(W0, np.float32), np.asarray(b0, np.float32),
                      np.asarray(bout, np.float32))

    n_steps = int(os.environ.get("DPH_KERNEL_STEPS", N_STEPS))
    db = _calibrate_full(state, init_noise, Wh_np, bh_np, Wout_np, tb, n_steps)

    if _cached.get("nc_steps") != n_steps:
        _cached["nc"] = _build_program(n_steps, tb["s_t"])
        _cached["nc_steps"] = n_steps
    nc = _cached["nc"]

    # ---- device-layout tables (shared across cores) ----
    # WTAB: [n_steps, 128, 256]; strip s rows 32s..32s+31 all hold W0A[t]
    wtab = np.empty((n_steps, 128, HIDDEN), np.float32)
    for s in range(4):
        wtab[:, s * 32 : (s + 1) * 32, :] = tb["W0A"][:n_steps]
    # BTAB: [n_steps, 128, 8]: cols 0-1 = e' chunks, 2..7 = bh+db (l, j)
    btab = np.empty((n_steps, 128, 8), np.float32)
    for j in range(2):
        btab[:, :, j] = tb["eprime"][:n_steps, j * 128 : (j + 1) * 128]
    bh_eff = bh_np[None, :, :] + db  # [n_steps, 3, 256]
    for l in range(3):
        for j in range(2):
            btab[:, :, 2 + l * 2 + j] = bh_eff[:, l, j * 128 : (j + 1) * 128]
    # WH8: [128, 3, 2, 2, 128] fp8: [p, l, j, k, m] = Wh[l][k*128+p, j*128+m]
    wh8 = np.empty((128, 3, 2, 2, 128), ml_dtypes.float8_e4m3)
    whq = np.asarray(Wh_np, ml_dtypes.float8_e4m3)
    for l in range(3):
        for j in range(2):
            for k in range(2):
                wh8[:, l, j, k, :] = whq[l, k * 128 : (k + 1) * 128,
                                         j * 128 : (j + 1) * 128]
    # WOUT: [128, 2, 32]: [p, c, m] = Wout[c*128+p, m]
    wout_dev = np.ascontiguousarray(
        Wout_np.reshape(2, 128, ACTION_DIM).transpose(1, 0, 2)
    )

    in_maps = []
    for c in range(N_CORES):
        rows = slice(c * B_CORE, (c + 1) * B_CORE)
        # zinit: [half, 32*strip+r, n] = noise[core*4096 + (half*4+strip)*512 + n, r]
        zin = (
            init_noise[rows]
            .reshape(N_HALF, 4, NB, ACTION_DIM)
            .transpose(0, 1, 3, 2)
            .reshape(N_HALF, 128, NB)
        )
        in_maps.append(
            {
                "stateT": np.ascontiguousarray(state[rows].T),
                "zinit": np.ascontiguousarray(zin),
                "WTAB": wtab,
                "BTAB": btab,
                "W0s": tb["W0s"],
                "WH8": wh8,
                "WOUT": wout_dev,
            }
        )

    _cached["in_maps"] = in_maps
    res = run_bass_kernel_spmd(nc, in_maps, core_ids=list(range(N_CORES)))
    _cached["last_results"] = res

    g50 = np.float32(tb["g_final"])
    beta50 = tb["beta_final"].astype(np.float32)
    out = np.empty((BATCH, ACTION_DIM), np.float32)
    for c in range(N_CORES):
        rows = slice(c * B_CORE, (c + 1) * B_CORE)
        oz = res.results[c]["outZ"].reshape(N_HALF, 4, ACTION_DIM, NB)
        out[rows] = (
            g50 * oz.transpose(0, 1, 3, 2).reshape(B_CORE, ACTION_DIM) + beta50
        )
    return out


if __name__ == "__main__":
    _c = np.load("/root/problem/ref_cache.npz")
    inputs = {k: _c[k] for k in _c.files if k != "expected"}
    got = kernel(**inputs)
    exp = _c["expected"]
    d = np.linalg.norm(got - exp) / np.linalg.norm(exp)
    print(f"L2 relative error: {d:.4e}")


# revision 4
# speedup vs baseline: 1.2268x; 1.2268x over previous
"""Trainium2 Bass kernel for nn_DiffusionPolicyHead (EDM/DDIM sampler head).

Strategy
--------
Pure data parallel over 8 NeuronCores (batch 32768 -> 4096/core).

Host-side algebra (per-step scalars fold into constants): with
    a_t = ratio + (1-ratio) c_skip,   b_t = (1-ratio) c_out,
the DDIM update is action' = a_t action + b_t (h3 @ Wout + bout).
Substituting action_t = g_t z_t + beta_t gives
    z_{t+1} = z_t + (b_t/g_{t+1}) * (h3 @ Wout),   z_0 = init_noise,
and layer 0 becomes  h0 = relu(z @ (c_in g W0a) + state @ W0s + e'_t)
with e'_t = emb_t @ W0e + b0 + c_in (beta_t @ W0a) a per-step bias vector
(rides the ACT/DVE bias slot -- free). Final action = g_50 z_50 + beta_50
applied on host.

Device layout (per core): feature-major [feat, batch]; batch 4096 = 2 halves
x 2 pairs x 2 blocks of N=512. z is [32, 2048] per half (blocks along free).
  - Layer 0: f32r; per (pair, j): two K=128 state matmuls + two K=32 z
    matmuls accumulate into a [128, 1024] 2-bank PSUM pair tile.
  - Hidden layers: fp8 e4m3 + DoubleRow -- K=256 in ONE matmul per
    (block, chunk) at ~1.4x bf16 rate. Per-(step,layer) bias corrections,
    calibrated on a 512-sample numpy run of the quantized pipeline, cancel
    the batch-coherent part of the fp8 quantization error.
  - Output layer: f32r M=32; per pair, 4 matmuls into po[0:32, 1024], then
    one fused z-update DVE op (z' = s_t*po + z) per pair.
Epilogues (relu+bias, fp32->fp8) merge 2 blocks per op ([128, 1024] across
two adjacent PSUM banks): ACT 10 ops / DVE 6 relu + 2 z-updates per
half-pass, balancing ~10us engine time against ~10.9us of PE work.
"""

import os
import sys

sys.path.insert(0, "/opt/trn_rl_repo")

import numpy as np
import ml_dtypes

BATCH, STATE_DIM, ACTION_DIM = 32768, 128, 32
HIDDEN, EMBED, N_STEPS = 256, 64, 50
SIGMA_MAX, SIGMA_MIN, RHO = 80.0, 0.001, 7.0
N_CORES = 8
B_CORE = BATCH // N_CORES  # 4096
NB = 512  # block columns (one PSUM bank of fp32)
N_HALF = 2  # halves per core; each half = 4 blocks in one [32, 2048] z tile
CAL_SAMPLES = 512

_cached = {}


def _q8(x):
    return np.asarray(x, ml_dtypes.float8_e4m3).astype(np.float32)


def _r11(x):
    """float32r as seen by the PE: mantissa 23->11 bits, RNE."""
    u = np.ascontiguousarray(np.asarray(x, np.float32)).view(np.uint32)
    half = np.uint32(1 << 11)
    u = u + (half - 1 + ((u >> 12) & 1))
    u &= np.uint32(0xFFFFF000)
    return u.view(np.float32)


def _host_tables(W0, b0, bout):
    """Fold per-step diffusion constants into weight tables (float64)."""
    W0 = W0.astype(np.float64)
    b0 = b0.astype(np.float64)
    bout = bout.astype(np.float64)
    W0a = W0[:ACTION_DIM]
    W0e = W0[ACTION_DIM : ACTION_DIM + EMBED]
    W0s = W0[ACTION_DIM + EMBED :]

    ramp = np.linspace(0.0, 1.0, N_STEPS)
    min_r, max_r = SIGMA_MIN ** (1.0 / RHO), SIGMA_MAX ** (1.0 / RHO)
    sig = np.concatenate([(max_r + ramp * (min_r - max_r)) ** RHO, np.zeros(1)])

    half = EMBED // 2
    freqs = np.exp(-np.log(10000.0) * np.arange(half, dtype=np.float64) / half)

    sd = 1.0
    g = sig[0]
    beta = np.zeros(ACTION_DIM)
    W0A = np.empty((N_STEPS, ACTION_DIM, HIDDEN))
    eprime = np.empty((N_STEPS, HIDDEN))
    s_t = np.empty(N_STEPS)
    for t in range(N_STEPS):
        s, sn = sig[t], sig[t + 1]
        var = s * s + sd * sd
        c_in = 1.0 / np.sqrt(var)
        c_skip = sd * sd / var
        c_out = s * sd / np.sqrt(var)
        ratio = sn / s
        a_t = ratio + (1.0 - ratio) * c_skip
        b_t = (1.0 - ratio) * c_out
        ang = np.log(s) * freqs
        emb = np.concatenate([np.sin(ang), np.cos(ang)])
        W0A[t] = c_in * g * W0a
        eprime[t] = emb @ W0e + b0 + c_in * (beta @ W0a)
        g_next = a_t * g
        beta = a_t * beta + b_t * bout
        s_t[t] = b_t / g_next
        g = g_next
    return dict(
        W0A=W0A.astype(np.float32),
        eprime=eprime.astype(np.float32),
        s_t=s_t.astype(np.float32),
        W0s=W0s.astype(np.float32),
        g_final=g,
        beta_final=beta,
    )


def _calibrate(state, init_noise, Wh, bh, Wout, tb, n_steps):
    """Per-(step, layer) bias corrections for fp8 hidden layers: batch-mean
    of (fp8 matmul - exact matmul) on a sample, emulating device rounding."""
    rng = np.random.default_rng(12345)
    idx = rng.choice(BATCH, CAL_SAMPLES, replace=False)
    st = state[idx].astype(np.float32)
    z = init_noise[idx].astype(np.float32).copy()
    u = _r11(st) @ _r11(tb["W0s"])
    Wh8 = _q8(Wh)
    Wout_r = _r11(Wout)
    db = np.zeros((n_steps, 3, HIDDEN), np.float32)
    for t in range(n_steps):
        h = np.maximum(_r11(z) @ _r11(tb["W0A"][t]) + u + tb["eprime"][t], 0.0)
        for l in range(3):
            A = _q8(h) @ Wh8[l]
            db[t, l] = -(A - h @ Wh[l]).mean(axis=0)
            h = np.maximum(A + bh[l] + db[t, l], 0.0)
        z = z + tb["s_t"][t] * (_r11(h) @ Wout_r)
    return db


def _build_program(n_steps, s_t):
    import concourse.bacc as bacc
    import concourse.mybir as mybir
    from concourse import tile
    from contextlib import ExitStack

    F32 = mybir.dt.float32
    F32R = mybir.dt.float32r
    F8 = mybir.dt.float8e4
    AF = mybir.ActivationFunctionType
    ALU = mybir.AluOpType
    DR = mybir.MatmulPerfMode.DoubleRow

    nc = bacc.Bacc("TRN2", target_bir_lowering=False, debug=False, num_devices=N_CORES)

    state_in = nc.declare_dram_parameter("stateT", [STATE_DIM, B_CORE], F32R, isOutput=False)
    zinit_in = nc.declare_dram_parameter("zinit", [N_HALF, ACTION_DIM, 4 * NB], F32R, isOutput=False)
    wtab_in = nc.declare_dram_parameter("WTAB", [n_steps, ACTION_DIM, HIDDEN], F32R, isOutput=False)
    btab_in = nc.declare_dram_parameter("BTAB", [n_steps, 128, 8], F32, isOutput=False)
    w0s_in = nc.declare_dram_parameter("W0s", [STATE_DIM, HIDDEN], F32R, isOutput=False)
    wh8_in = nc.declare_dram_parameter("WH8", [128, 3, 2, 2, 128], F8, isOutput=False)
    wout_in = nc.declare_dram_parameter("WOUT", [128, 2, ACTION_DIM], F32R, isOutput=False)
    out_ext = nc.declare_dram_parameter("outZ", [N_HALF, ACTION_DIM, 4 * NB], F32R, isOutput=True)

    with tile.TileContext(nc) as tc:
        with ExitStack() as ctx:
            wpool = ctx.enter_context(tc.tile_pool(name="weights", bufs=1))
            zpool = ctx.enter_context(tc.tile_pool(name="zbufs", bufs=1))
            hpool = ctx.enter_context(tc.tile_pool(name="acts8", bufs=6))
            h3pool = ctx.enter_context(tc.tile_pool(name="acts3", bufs=4))
            wstream = ctx.enter_context(tc.tile_pool(name="wstream", bufs=4))
            bstream = ctx.enter_context(tc.tile_pool(name="bstream", bufs=4))
            ppool = ctx.enter_context(tc.tile_pool(name="psum", bufs=4, space="PSUM"))

            stateT = wpool.tile([STATE_DIM, B_CORE], F32R, tag="stateT")
            w0s = wpool.tile([STATE_DIM, HIDDEN], F32R, tag="w0s")
            wh8 = wpool.tile([128, 3, 2, 2, 128], F8, tag="wh8")
            wout = wpool.tile([128, 2, ACTION_DIM], F32R, tag="wout")
            for b in range(8):
                nc.sync.dma_start(
                    stateT[:, b * NB : (b + 1) * NB],
                    state_in[:, b * NB : (b + 1) * NB],
                )
            nc.sync.dma_start(w0s[:], w0s_in[:])
            nc.sync.dma_start(wh8[:], wh8_in[:])
            nc.sync.dma_start(wout[:], wout_in[:])

            # z ping-pong: [2 parity][2 halves] tiles [32, 2048], 4 blocks
            # along free dim. Only parity 0 needs init (parity 1 is fully
            # written by step 0's z-updates).
            zt = [
                [
                    zpool.tile([ACTION_DIM, 4 * NB], F32R, tag=f"z{p}_{h}", name=f"z{p}_{h}")
                    for h in range(N_HALF)
                ]
                for p in range(2)
            ]
            for h in range(N_HALF):
                nc.sync.dma_start(zt[0][h][:], zinit_in[h])

            for t in range(n_steps):
                wtab = wstream.tile([ACTION_DIM, HIDDEN], F32R, tag="wtab", name="wtab")
                btab = bstream.tile([128, 8], F32, tag="btab", name="btab")
                nc.sync.dma_start(wtab[:], wtab_in[t])
                nc.sync.dma_start(btab[:], btab_in[t])

                for half in range(N_HALF):
                    zc, zn = zt[t % 2][half], zt[(t + 1) % 2][half]
                    h_cur = []  # per-pair tiles of current layer
                    # ---- layer 0 (f32r: K=128 state + K=32 z) ----
                    for pair in range(2):
                        h0 = hpool.tile(
                            [128, 2, 2, NB], F8, tag=f"h0_{pair}", name=f"h0_{pair}"
                        )
                        for j in range(2):
                            jsl = slice(j * 128, (j + 1) * 128)
                            pt = ppool.tile([128, 2 * NB], F32, tag="ps", name="p0")
                            for bp in range(2):
                                gcol = (half * 4 + pair * 2 + bp) * NB
                                nc.tensor.matmul(
                                    pt[:, bp * NB : (bp + 1) * NB],
                                    w0s[:, jsl],
                                    stateT[:, gcol : gcol + NB],
                                    start=True,
                                    stop=False,
                                )
                            for bp in range(2):
                                lcol = (pair * 2 + bp) * NB
                                nc.tensor.matmul(
                                    pt[:, bp * NB : (bp + 1) * NB],
                                    wtab[:, jsl],
                                    zc[:, lcol : lcol + NB],
                                    start=False,
                                    stop=True,
                                )
                            bias_ap = btab[:, j : j + 1]
                            if j == 0:
                                nc.scalar.activation(
                                    h0[:, :, j, :], pt[:], AF.Relu, bias=bias_ap
                                )
                            else:
                                nc.vector.tensor_scalar(
                                    h0[:, :, j, :], pt[:], bias_ap, 0.0, ALU.add, ALU.max
                                )
                        h_cur.append(h0)
                    # ---- hidden layers (fp8 DoubleRow, K=256 per matmul) ----
                    for l in range(3):
                        last = l == 2
                        h_next = []
                        for pair in range(2):
                            if last:
                                hn = h3pool.tile(
                                    [128, 2, 2, NB], F32R, tag=f"h3_{pair}", name=f"h3_{pair}"
                                )
                            else:
                                hn = hpool.tile(
                                    [128, 2, 2, NB], F8, tag=f"h{l + 1}_{pair}", name=f"h{l + 1}_{pair}"
                                )
                            for j in range(2):
                                pt = ppool.tile([128, 2 * NB], F32, tag="ps", name="pl")
                                for bp in range(2):
                                    nc.tensor.matmul(
                                        pt[:, bp * NB : (bp + 1) * NB],
                                        wh8[:, l, j, :, :],
                                        h_cur[pair][:, bp, :, :],
                                        start=True,
                                        stop=True,
                                        perf_mode=DR,
                                    )
                                bias_ap = btab[:, 2 + l * 2 + j : 3 + l * 2 + j]
                                # ACT 10 / DVE 6 relu ops per half-pass:
                                # j=0 on ACT, j=1 on DVE, except L3 j=1 on ACT.
                                if j == 0 or last:
                                    nc.scalar.activation(
                                        hn[:, :, j, :], pt[:], AF.Relu, bias=bias_ap
                                    )
                                else:
                                    nc.vector.tensor_scalar(
                                        hn[:, :, j, :], pt[:], bias_ap, 0.0, ALU.add, ALU.max
                                    )
                            h_next.append(hn)
                        h_cur = h_next
                    # ---- output layer (f32r M=32) + fused z-update ----
                    for pair in range(2):
                        po = ppool.tile([128, 2 * NB], F32, tag="ps", name="po")
                        for bp in range(2):
                            for c in range(2):
                                nc.tensor.matmul(
                                    po[:ACTION_DIM, bp * NB : (bp + 1) * NB],
                                    wout[:, c, :],
                                    h_cur[pair][:, bp, c, :],
                                    start=(c == 0),
                                    stop=(c == 1),
                                )
                        psl = slice(pair * 2 * NB, (pair + 1) * 2 * NB)
                        nc.vector.scalar_tensor_tensor(
                            zn[:, psl],
                            po[:ACTION_DIM, :],
                            float(s_t[t]),
                            zc[:, psl],
                            ALU.mult,
                            ALU.add,
                        )

            zfin = zt[n_steps % 2]
            for h in range(N_HALF):
                nc.sync.dma_start(out_ext[h], zfin[h][:, :])

    nc.compile()
    return nc


def kernel(state, init_noise, W0, b0, Wh, bh, Wout, bout):
    from concourse.bass_utils import run_bass_kernel_spmd

    state = np.ascontiguousarray(np.asarray(state, np.float32))
    init_noise = np.ascontiguousarray(np.asarray(init_noise, np.float32))
    Wh_np = np.asarray(Wh, np.float32)
    bh_np = np.asarray(bh, np.float32)
    Wout_np = np.asarray(Wout, np.float32)

    tb = _host_tables(np.asarray(W0, np.float32), np.asarray(b0, np.float32),
                      np.asarray(bout, np.float32))

    n_steps = int(os.environ.get("DPH_KERNEL_STEPS", N_STEPS))
    db = _calibrate(state, init_noise, Wh_np, bh_np, Wout_np, tb, n_steps)

    if _cached.get("nc_steps") != n_steps:
        _cached["nc"] = _build_program(n_steps, tb["s_t"])
        _cached["nc_steps"] = n_steps
    nc = _cached["nc"]

    # ---- device-layout tables (shared across cores) ----
    wtab = np.ascontiguousarray(tb["W0A"][:n_steps])  # [n_steps, 32, 256]
    # BTAB: [n_steps, 128, 8]: cols 0-1 = e' chunks, 2..7 = bh+db (l, j)
    btab = np.empty((n_steps, 128, 8), np.float32)
    for j in range(2):
        btab[:, :, j] = tb["eprime"][:n_steps, j * 128 : (j + 1) * 128]
    bh_eff = bh_np[None, :, :] + db  # [n_steps, 3, 256]
    for l in range(3):
        for j in range(2):
            btab[:, :, 2 + l * 2 + j] = bh_eff[:, l, j * 128 : (j + 1) * 128]
    # WH8: [128, 3, 2, 2, 128] fp8: [p, l, j, k, m] = Wh[l][k*128+p, j*128+m]
    wh8 = np.empty((128, 3, 2, 2, 128), ml_dtypes.float8_e4m3)
    whq = np.asarray(Wh_np, ml_dtypes.float8_e4m3)
    for l in range(3):
        for j in range(2):
            for k in range(2):
                wh8[:, l, j, k, :] = whq[l, k * 128 : (k + 1) * 128,
                                         j * 128 : (j + 1) * 128]
    # WOUT: [128, 2, 32]: [p, c, m] = Wout[c*128+p, m]
    wout_dev = np.ascontiguousarray(
        Wout_np.reshape(2, 128, ACTION_DIM).transpose(1, 0, 2)
    )

    in_maps = []
    for c in range(N_CORES):
        rows = slice(c * B_CORE, (c + 1) * B_CORE)
        # zinit: [half, r, blk*512+n] = noise[core*4096 + (half*4+blk)*512 + n, r]
        zin = (
            init_noise[rows]
            .reshape(N_HALF, 4 * NB, ACTION_DIM)
            .transpose(0, 2, 1)
        )
        in_maps.append(
            {
                "stateT": np.ascontiguousarray(state[rows].T),
                "zinit": np.ascontiguousarray(zin),
                "WTAB": wtab,
                "BTAB": btab,
                "W0s": tb["W0s"],
                "WH8": wh8,
                "WOUT": wout_dev,
            }
        )

    _cached["in_maps"] = in_maps
    res = run_bass_kernel_spmd(nc, in_maps, core_ids=list(range(N_CORES)))
    _cached["last_results"] = res

    g50 = np.float32(tb["g_final"])
    beta50 = tb["beta_final"].astype(np.float32)
    out = np.empty((BATCH, ACTION_DIM), np.float32)
    for c in range(N_CORES):
        rows = slice(c * B_CORE, (c + 1) * B_CORE)
        oz = res.results[c]["outZ"]  # [half, 32, 2048]
        out[rows] = g50 * oz.transpose(0, 2, 1).reshape(B_CORE, ACTION_DIM) + beta50
    return out


if __name__ == "__main__":
    _c = np.load("/root/problem/ref_cache.npz")
    inputs = {k: _c[k] for k in _c.files if k != "expected"}
    got = kernel(**inputs)
    exp = _c["expected"]
    d = np.linalg.norm(got - exp) / np.linalg.norm(exp)
    print(f"L2 relative error: {d:.4e}")


# revision 19
# speedup vs baseline: 1.7219x; 1.4036x over previous
"""Trainium2 Bass kernel for nn_DiffusionPolicyHead (EDM/DDIM sampler head).

Strategy
--------
Pure data parallel over 8 NeuronCores (batch 32768 -> 4096/core).

Host-side algebra (per-step scalars fold into constants): with
    a_t = ratio + (1-ratio) c_skip,   b_t = (1-ratio) c_out,
the DDIM update is action' = a_t action + b_t (h3 @ Wout + bout).
Substituting action_t = g_t z_t + beta_t gives
    z_{t+1} = z_t + (b_t/g_{t+1}) * (h3 @ Wout),   z_0 = init_noise,
and layer 0 becomes  h0 = relu(z @ (c_in g W0a) + state @ W0s + e'_t)
with e'_t = emb_t @ W0e + b0 + c_in (beta_t @ W0a) a per-step bias vector
(rides the ACT/DVE bias slot -- free). Final action = g_50 z_50 + beta_50
applied on host.

Device layout (per core): feature-major [feat, batch]; batch 4096 = 2 halves
x 2 pairs x 2 blocks of N=512. z is [32, 2048] per half (blocks along free).
  - Layer 0: f32r; per (pair, j): two K=128 state matmuls + two K=32 z
    matmuls accumulate into a [128, 1024] 2-bank PSUM pair tile.
  - Hidden layers: fp8 e4m3 + DoubleRow -- K=256 in ONE matmul per
    (block, chunk) at ~1.4x bf16 rate. Per-(step,layer) bias corrections,
    calibrated on a 512-sample numpy run of the quantized pipeline, cancel
    the batch-coherent part of the fp8 quantization error.
  - Output layer: f32r M=32; per pair, 4 matmuls into po[0:32, 1024], then
    one fused z-update DVE op (z' = s_t*po + z) per pair.
Epilogues (relu+bias, fp32->fp8) merge 2 blocks per op ([128, 1024] across
two adjacent PSUM banks): ACT 10 ops / DVE 6 relu + 2 z-updates per
half-pass, balancing ~10us engine time against ~10.9us of PE work.
"""

import os
import sys

sys.path.insert(0, "/opt/trn_rl_repo")

import numpy as np
import ml_dtypes

BATCH, STATE_DIM, ACTION_DIM = 32768, 128, 32
HIDDEN, EMBED, N_STEPS = 256, 64, 50
SIGMA_MAX, SIGMA_MIN, RHO = 80.0, 0.001, 7.0
N_CORES = 8
B_CORE = BATCH // N_CORES  # 4096
NB = 512  # block columns (one PSUM bank of fp32)
N_HALF = 2  # halves per core; each half = 4 blocks in one [32, 2048] z tile
CAL_SAMPLES = 512

_cached = {}


def _q8(x):
    return np.asarray(x, ml_dtypes.float8_e4m3).astype(np.float32)


def _r11(x):
    """float32r as seen by the PE: mantissa 23->11 bits, RNE."""
    u = np.ascontiguousarray(np.asarray(x, np.float32)).view(np.uint32)
    half = np.uint32(1 << 11)
    u = u + (half - 1 + ((u >> 12) & 1))
    u &= np.uint32(0xFFFFF000)
    return u.view(np.float32)


def _host_tables(W0, b0, bout):
    """Fold per-step diffusion constants into weight tables (float64)."""
    W0 = W0.astype(np.float64)
    b0 = b0.astype(np.float64)
    bout = bout.astype(np.float64)
    W0a = W0[:ACTION_DIM]
    W0e = W0[ACTION_DIM : ACTION_DIM + EMBED]
    W0s = W0[ACTION_DIM + EMBED :]

    ramp = np.linspace(0.0, 1.0, N_STEPS)
    min_r, max_r = SIGMA_MIN ** (1.0 / RHO), SIGMA_MAX ** (1.0 / RHO)
    sig = np.concatenate([(max_r + ramp * (min_r - max_r)) ** RHO, np.zeros(1)])

    half = EMBED // 2
    freqs = np.exp(-np.log(10000.0) * np.arange(half, dtype=np.float64) / half)

    sd = 1.0
    g = sig[0]
    beta = np.zeros(ACTION_DIM)
    W0A = np.empty((N_STEPS, ACTION_DIM, HIDDEN))
    eprime = np.empty((N_STEPS, HIDDEN))
    s_t = np.empty(N_STEPS)
    for t in range(N_STEPS):
        s, sn = sig[t], sig[t + 1]
        var = s * s + sd * sd
        c_in = 1.0 / np.sqrt(var)
        c_skip = sd * sd / var
        c_out = s * sd / np.sqrt(var)
        ratio = sn / s
        a_t = ratio + (1.0 - ratio) * c_skip
        b_t = (1.0 - ratio) * c_out
        ang = np.log(s) * freqs
        emb = np.concatenate([np.sin(ang), np.cos(ang)])
        W0A[t] = c_in * g * W0a
        eprime[t] = emb @ W0e + b0 + c_in * (beta @ W0a)
        g_next = a_t * g
        beta = a_t * beta + b_t * bout
        s_t[t] = b_t / g_next
        g = g_next
    return dict(
        W0A=W0A.astype(np.float32),
        eprime=eprime.astype(np.float32),
        s_t=s_t.astype(np.float32),
        W0s=W0s.astype(np.float32),
        g_final=g,
        beta_final=beta,
    )


def _calibrate(state, init_noise, Wh, bh, Wout, tb, n_steps):
    """Per-(step, layer) bias corrections for fp8 hidden layers: batch-mean
    of (fp8 matmul - exact matmul) on a sample, emulating device rounding."""
    rng = np.random.default_rng(12345)
    idx = rng.choice(BATCH, CAL_SAMPLES, replace=False)
    st = state[idx].astype(np.float32)
    z = init_noise[idx].astype(np.float32).copy()
    u = _r11(st) @ _r11(tb["W0s"])
    Wh8 = _q8(Wh)
    Wout_r = _r11(Wout)
    db = np.zeros((n_steps, 3, HIDDEN), np.float32)
    for t in range(n_steps):
        h = np.maximum(_r11(z) @ _r11(tb["W0A"][t]) + u + tb["eprime"][t], 0.0)
        for l in range(3):
            A = _q8(h) @ Wh8[l]
            db[t, l] = -(A - h @ Wh[l]).mean(axis=0)
            h = np.maximum(A + bh[l] + db[t, l], 0.0)
        z = z + tb["s_t"][t] * (_r11(h) @ Wout_r)
    return db


def _build_program(n_steps, s_t):
    import concourse.bacc as bacc
    import concourse.mybir as mybir
    from concourse import tile
    from contextlib import ExitStack

    F32 = mybir.dt.float32
    F32R = mybir.dt.float32r
    F8 = mybir.dt.float8e4
    BF16 = mybir.dt.bfloat16
    AF = mybir.ActivationFunctionType
    ALU = mybir.AluOpType
    DR = mybir.MatmulPerfMode.DoubleRow

    nc = bacc.Bacc("TRN2", target_bir_lowering=False, debug=False, num_devices=N_CORES)

    state_in = nc.declare_dram_parameter("stateT", [STATE_DIM, B_CORE], F32R, isOutput=False)
    zinit_in = nc.declare_dram_parameter("zinit", [N_HALF, 128, 4 * NB], F32R, isOutput=False)
    # K=32 matmuls measure ~2x a full matmul on this part; pad the z
    # contraction to K=128 with zero weight rows (z tile rows 32-127 are
    # junk but multiply by zero).
    wtab_in = nc.declare_dram_parameter("WTAB", [n_steps, 128, HIDDEN], F32R, isOutput=False)
    btab_in = nc.declare_dram_parameter("BTAB", [n_steps, 128, 8], F32, isOutput=False)
    w0s_in = nc.declare_dram_parameter("W0s", [STATE_DIM, HIDDEN], F32R, isOutput=False)
    wh8_in = nc.declare_dram_parameter("WH8", [128, 3, 2, 2, 128], F8, isOutput=False)
    wout_in = nc.declare_dram_parameter("WOUT", [128, 2, ACTION_DIM], BF16, isOutput=False)
    out_ext = nc.declare_dram_parameter("outZ", [N_HALF, ACTION_DIM, 4 * NB], F32R, isOutput=True)

    with tile.TileContext(nc) as tc:
        with ExitStack() as ctx:
            wpool = ctx.enter_context(tc.tile_pool(name="weights", bufs=1))
            zpool = ctx.enter_context(tc.tile_pool(name="zbufs", bufs=1))
            hpool = ctx.enter_context(tc.tile_pool(name="acts8", bufs=6))
            h3pool = ctx.enter_context(tc.tile_pool(name="acts3", bufs=6))
            wstream = ctx.enter_context(tc.tile_pool(name="wstream", bufs=4))
            bstream = ctx.enter_context(tc.tile_pool(name="bstream", bufs=4))
            ppool = ctx.enter_context(tc.tile_pool(name="psum", bufs=4, space="PSUM"))

            stateT = wpool.tile([STATE_DIM, B_CORE], F32R, tag="stateT")
            w0s = wpool.tile([STATE_DIM, HIDDEN], F32R, tag="w0s")
            wh8 = wpool.tile([128, 3, 2, 2, 128], F8, tag="wh8")
            wout = wpool.tile([128, 2, ACTION_DIM], BF16, tag="wout")
            for b in range(8):
                nc.sync.dma_start(
                    stateT[:, b * NB : (b + 1) * NB],
                    state_in[:, b * NB : (b + 1) * NB],
                )
            nc.sync.dma_start(w0s[:], w0s_in[:])
            nc.sync.dma_start(wh8[:], wh8_in[:])
            nc.sync.dma_start(wout[:], wout_in[:])

            # z ping-pong: [2 parity][2 halves] tiles [128, 2048], 4 blocks
            # along free dim; only rows 0-31 carry z (rows 32-127 are read
            # by the padded K=128 z matmuls against zero weights — memset
            # once so they're initialized). Only parity-0 rows 0-31 need
            # the noise init (parity 1 is fully written by step 0).
            zt = [
                [
                    zpool.tile([128, 4 * NB], F32R, tag=f"z{p}_{h}", name=f"z{p}_{h}")
                    for h in range(N_HALF)
                ]
                for p in range(2)
            ]
            for p in range(2):
                for h in range(N_HALF):
                    nc.sync.dma_start(zt[p][h][:], zinit_in[h])

            for t in range(n_steps):
                wtab = wstream.tile([128, HIDDEN], F32R, tag="wtab", name="wtab")
                btab = bstream.tile([128, 8], F32, tag="btab", name="btab")
                nc.sync.dma_start(wtab[:], wtab_in[t])
                nc.sync.dma_start(btab[:], btab_in[t])

                # Units interleave the two independent halves so the PE
                # always has ready matmuls while the other half's epilogues
                # drain.
                units = [(0, 0), (1, 0), (0, 1), (1, 1)]  # (half, pair)
                zc = [zt[t % 2][h] for h in range(N_HALF)]
                zn = [zt[(t + 1) % 2][h] for h in range(N_HALF)]
                h_cur = {}
                # ---- layer 0 (f32r: K=128 state + zero-padded K=128 z) ----
                for half, pair in units:
                    h0 = hpool.tile(
                        [128, 2, 2, NB], F8, tag="h0", name=f"h0_{half}_{pair}"
                    )
                    for j in range(2):
                        jsl = slice(j * 128, (j + 1) * 128)
                        pt = ppool.tile([128, 2 * NB], F32, tag="ps", name="p0")
                        for bp in range(2):
                            gcol = (half * 4 + pair * 2 + bp) * NB
                            nc.tensor.matmul(
                                pt[:, bp * NB : (bp + 1) * NB],
                                w0s[:, jsl],
                                stateT[:, gcol : gcol + NB],
                                start=True,
                                stop=False,
                            )
                        for bp in range(2):
                            lcol = (pair * 2 + bp) * NB
                            nc.tensor.matmul(
                                pt[:, bp * NB : (bp + 1) * NB],
                                wtab[:, jsl],
                                zc[half][:, lcol : lcol + NB],
                                start=False,
                                stop=True,
                            )
                        bias_ap = btab[:, j : j + 1]
                        if j == 0:
                            nc.scalar.activation(
                                h0[:, :, j, :], pt[:], AF.Relu, bias=bias_ap
                            )
                        else:
                            nc.vector.tensor_scalar(
                                h0[:, :, j, :], pt[:], bias_ap, 0.0, ALU.add, ALU.max
                            )
                    h_cur[(half, pair)] = h0
                # ---- hidden layers (fp8 DoubleRow, K=256 per matmul) ----
                for l in range(3):
                    last = l == 2
                    for half, pair in units:
                        if last:
                            hn = h3pool.tile(
                                [128, 2, 2, NB], BF16, tag="h3", name=f"h3_{half}_{pair}"
                            )
                        else:
                            hn = hpool.tile(
                                [128, 2, 2, NB], F8, tag=f"h{l + 1}", name=f"h{l + 1}_{half}_{pair}"
                            )
                        for j in range(2):
                            pt = ppool.tile([128, 2 * NB], F32, tag="ps", name="pl")
                            for bp in range(2):
                                nc.tensor.matmul(
                                    pt[:, bp * NB : (bp + 1) * NB],
                                    wh8[:, l, j, :, :],
                                    h_cur[(half, pair)][:, bp, :, :],
                                    start=True,
                                    stop=True,
                                    perf_mode=DR,
                                )
                            bias_ap = btab[:, 2 + l * 2 + j : 3 + l * 2 + j]
                            # ACT 10 / DVE 6 relu ops per half-pass:
                            # j=0 on ACT, j=1 on DVE, except L3 j=1 on ACT.
                            if j == 0 or last:
                                nc.scalar.activation(
                                    hn[:, :, j, :], pt[:], AF.Relu, bias=bias_ap
                                )
                            else:
                                nc.vector.tensor_scalar(
                                    hn[:, :, j, :], pt[:], bias_ap, 0.0, ALU.add, ALU.max
                                )
                        h_cur[(half, pair)] = hn
                # ---- output layer (bf16 M=32) + fused z-update ----
                for half, pair in units:
                    po = ppool.tile([128, 2 * NB], F32, tag="ps", name="po")
                    for bp in range(2):
                        for c in range(2):
                            nc.tensor.matmul(
                                po[:ACTION_DIM, bp * NB : (bp + 1) * NB],
                                wout[:, c, :],
                                h_cur[(half, pair)][:, bp, c, :],
                                start=(c == 0),
                                stop=(c == 1),
                            )
                    psl = slice(pair * 2 * NB, (pair + 1) * 2 * NB)
                    nc.vector.scalar_tensor_tensor(
                        zn[half][:ACTION_DIM, psl],
                        po[:ACTION_DIM, :],
                        float(s_t[t]),
                        zc[half][:ACTION_DIM, psl],
                        ALU.mult,
                        ALU.add,
                    )

            zfin = zt[n_steps % 2]
            for h in range(N_HALF):
                nc.sync.dma_start(out_ext[h], zfin[h][:ACTION_DIM, :])

    nc.compile()
    return nc


def kernel(state, init_noise, W0, b0, Wh, bh, Wout, bout):
    from concourse.bass_utils import run_bass_kernel_spmd

    state = np.ascontiguousarray(np.asarray(state, np.float32))
    init_noise = np.ascontiguousarray(np.asarray(init_noise, np.float32))
    Wh_np = np.asarray(Wh, np.float32)
    bh_np = np.asarray(bh, np.float32)
    Wout_np = np.asarray(Wout, np.float32)

    tb = _host_tables(np.asarray(W0, np.float32), np.asarray(b0, np.float32),
                      np.asarray(bout, np.float32))

    n_steps = int(os.environ.get("DPH_KERNEL_STEPS", N_STEPS))
    db = _calibrate(state, init_noise, Wh_np, bh_np, Wout_np, tb, n_steps)

    if _cached.get("nc_steps") != n_steps:
        _cached["nc"] = _build_program(n_steps, tb["s_t"])
        _cached["nc_steps"] = n_steps
    nc = _cached["nc"]

    # ---- device-layout tables (shared across cores) ----
    # rows 32-127 zero: padding for the K=128 z matmuls
    wtab = np.zeros((n_steps, 128, HIDDEN), np.float32)
    wtab[:, :ACTION_DIM, :] = tb["W0A"][:n_steps]
    # BTAB: [n_steps, 128, 8]: cols 0-1 = e' chunks, 2..7 = bh+db (l, j)
    btab = np.empty((n_steps, 128, 8), np.float32)
    for j in range(2):
        btab[:, :, j] = tb["eprime"][:n_steps, j * 128 : (j + 1) * 128]
    bh_eff = bh_np[None, :, :] + db  # [n_steps, 3, 256]
    for l in range(3):
        for j in range(2):
            btab[:, :, 2 + l * 2 + j] = bh_eff[:, l, j * 128 : (j + 1) * 128]
    # WH8: [128, 3, 2, 2, 128] fp8: [p, l, j, k, m] = Wh[l][k*128+p, j*128+m]
    wh8 = np.empty((128, 3, 2, 2, 128), ml_dtypes.float8_e4m3)
    whq = np.asarray(Wh_np, ml_dtypes.float8_e4m3)
    for l in range(3):
        for j in range(2):
            for k in range(2):
                wh8[:, l, j, k, :] = whq[l, k * 128 : (k + 1) * 128,
                                         j * 128 : (j + 1) * 128]
    # WOUT: [128, 2, 32] bf16: [p, c, m] = Wout[c*128+p, m]
    wout_dev = np.ascontiguousarray(
        Wout_np.reshape(2, 128, ACTION_DIM).transpose(1, 0, 2)
    ).astype(ml_dtypes.bfloat16)

    in_maps = []
    for c in range(N_CORES):
        rows = slice(c * B_CORE, (c + 1) * B_CORE)
        # zinit: [half, r, blk*512+n] = noise[core*4096 + (half*4+blk)*512 + n, r]
        # rows 32-127 zero (padding read by the K=128 z matmuls)
        zin = np.zeros((N_HALF, 128, 4 * NB), np.float32)
        zin[:, :ACTION_DIM, :] = (
            init_noise[rows]
            .reshape(N_HALF, 4 * NB, ACTION_DIM)
            .transpose(0, 2, 1)
        )
        in_maps.append(
            {
                "stateT": np.ascontiguousarray(state[rows].T),
                "zinit": np.ascontiguousarray(zin),
                "WTAB": wtab,
                "BTAB": btab,
                "W0s": tb["W0s"],
                "WH8": wh8,
                "WOUT": wout_dev,
            }
        )

    _cached["in_maps"] = in_maps
    res = run_bass_kernel_spmd(nc, in_maps, core_ids=list(range(N_CORES)))
    _cached["last_results"] = res

    g50 = np.float32(tb["g_final"])
    beta50 = tb["beta_final"].astype(np.float32)
    out = np.empty((BATCH, ACTION_DIM), np.float32)
    for c in range(N_CORES):
        rows = slice(c * B_CORE, (c + 1) * B_CORE)
        oz = res.results[c]["outZ"]  # [half, 32, 2048]
        out[rows] = g50 * oz.transpose(0, 2, 1).reshape(B_CORE, ACTION_DIM) + beta50
    return out


if __name__ == "__main__":
    _c = np.load("/root/problem/ref_cache.npz")
    inputs = {k: _c[k] for k in _c.files if k != "expected"}
    got = kernel(**inputs)
    exp = _c["expected"]
    d = np.linalg.norm(got - exp) / np.linalg.norm(exp)
    print(f"L2 relative error: {d:.4e}")
